# revision 18
# baseline (speedup 1.0000x reference)
"""CRF loss (forward-algorithm NLL) on 8 Trainium2 NeuronCores.

Strategy: data-parallel over batch (8 sequences per core). The T-step
log-alpha recurrence runs in the exp domain so each step is a plain
matmul against exp(P) on the TensorEngine:

    a_t[j,b] = em_t[j,b] * sum_i expP[i,j] * a_{t-1}[i,b]

Emissions carry a constant shift exp(logit - ln(256*e)) so the state
magnitude stays near 1; an exact per-8-step rescale by the column sum
(via a ones-matmul, tracked in log space) absorbs the drift. The [j,b]
layout is preserved step to step (matmul output partitions = next
contraction partitions), so the scan needs no per-step transposes.
Emissions are exp'ed in bulk on the Scalar engine and transposed
chunkwise with DMA transpose into per-chunk fresh SBUF tiles (no slot
recycling -> every DMA carries at most the single sync-wait the ISA
allows). The gold-path score uses indirect-DMA gathers with host-
computed flat indices; cross-engine joins go through single-wait
"touch" ops so no instruction ever needs two semaphore waits.
"""

import os
import sys

import numpy as np

sys.path.insert(0, "/opt/trn_rl_repo")
os.environ.setdefault("MYCRO_LOCAL_CACHE", "1")

import concourse.bass as bass
import concourse.bacc as bacc
import concourse.mybir as mybir
from concourse.tile import TileContext

B, T, V = 64, 1024, 256
NCORES = 8
BS = B // NCORES          # 8 sequences per core
CT = 16                   # timesteps per emission chunk
RESC = 64                 # rescale period (steps)
C_SHIFT = 6.545177444479562  # ln(256*e); cancels expected per-step growth

f32 = mybir.dt.float32
fp8 = mybir.dt.float8e4
bf16 = mybir.dt.bfloat16
i32 = mybir.dt.int32
AF = mybir.ActivationFunctionType
ALU = mybir.AluOpType
AX = mybir.AxisListType


def build(t_steps=T):
    ch = t_steps // CT            # emission chunks
    gcols = t_steps // 16         # gather columns (16 groups per seq)

    nc = bacc.Bacc("TRN2")
    lg = nc.dram_tensor("lg", [BS, t_steps, V], f32, kind="ExternalInput")
    Pm = nc.dram_tensor("Pm", [V, V], f32, kind="ExternalInput")
    Sv = nc.dram_tensor("Sv", [1, V], f32, kind="ExternalInput")
    Ev = nc.dram_tensor("Ev", [1, V], f32, kind="ExternalInput")
    emidx = nc.dram_tensor("emidx", [128, gcols], i32, kind="ExternalInput")
    tridx = nc.dram_tensor("tridx", [128, gcols], i32, kind="ExternalInput")
    sidx = nc.dram_tensor("sidx", [BS, 1], i32, kind="ExternalInput")
    eidx = nc.dram_tensor("eidx", [BS, 1], i32, kind="ExternalInput")
    out = nc.dram_tensor("out", [1, 1], f32, kind="ExternalOutput")

    with TileContext(nc) as tc:
        with (
            tc.tile_pool(name="const", bufs=1) as cpool,
            tc.tile_pool(name="lraw", bufs=16) as lraw_pool,
            tc.tile_pool(name="lexp", bufs=16) as lexp_pool,
            tc.tile_pool(name="emt", bufs=16) as emt_pool,
            tc.tile_pool(name="a", bufs=t_steps + 140) as a_pool,
            tc.tile_pool(name="small", bufs=4) as spool,
            tc.tile_pool(name="rs", bufs=4) as rs_pool,
            tc.tile_pool(name="tch", bufs=2) as tpool,
            tc.tile_pool(name="ps", bufs=3, space="PSUM") as ps_pool,
            tc.tile_pool(name="fin", bufs=1, space="PSUM") as fin_pool,
            tc.tile_pool(name="junk", bufs=1, space="PSUM") as junk_pool,
        ):
            # ---- preamble: constants -------------------------------------
            praw = [cpool.tile([128, 256], f32, tag=f"praw{k}", name=f"praw{k}")
                    for k in range(2)]
            for k in range(2):
                nc.sync.dma_start(praw[k][:], Pm[k * 128:(k + 1) * 128, :])
            # PB[k][j] = exp(P[i-half k, j-half j]) in bf16
            PB = [[cpool.tile([128, 128], fp8, tag=f"pb{k}{j}", name=f"pb{k}{j}")
                   for j in range(2)] for k in range(2)]
            for k in range(2):
                for j in range(2):
                    nc.scalar.activation(
                        PB[k][j][:], praw[k][:, j * 128:(j + 1) * 128], AF.Exp)
            # emission shift, produced on ACT so the chunk exps join on one sem
            cshift = cpool.tile([128, 1], f32, tag="cshift")
            nc.scalar.activation(cshift[:], praw[0][:, 0:1], AF.Copy,
                                 bias=-C_SHIFT, scale=0.0)

            # exp(S), exp(E) as per-partition scalars [128,1] x2 (ACT)
            expS = [cpool.tile([128, 1], f32, tag=f"es{k}", name=f"es{k}")
                    for k in range(2)]
            expE = [cpool.tile([128, 1], f32, tag=f"ee{k}", name=f"ee{k}")
                    for k in range(2)]
            for k in range(2):
                svk = Sv[:].rearrange("a (p f) -> a p f", f=1)[0, k * 128:(k + 1) * 128]
                evk = Ev[:].rearrange("a (p f) -> a p f", f=1)[0, k * 128:(k + 1) * 128]
                tmpS = spool.tile([128, 1], f32, tag="tmpv")
                tmpE = spool.tile([128, 1], f32, tag="tmpv")
                nc.sync.dma_start(tmpS[:], svk)
                nc.sync.dma_start(tmpE[:], evk)
                nc.scalar.activation(expS[k][:], tmpS[:], AF.Exp)
                nc.scalar.activation(expE[k][:], tmpE[:], AF.Exp)

            ones_w = cpool.tile([128, 128], bf16, tag="ones")
            nc.vector.memset(ones_w[:], 1.0)
            acc_log = cpool.tile([1, 8], f32, tag="acc")
            nc.vector.memset(acc_log[:], float(t_steps) * C_SHIFT)

            # warm-up matmul: advances PE's view of ACT past the PB exps so
            # the scan matmuls each carry a single (DVE) wait
            warm_ps = junk_pool.tile([128, 64], f32, tag="junk")
            nc.tensor.matmul(warm_ps[:], PB[0][0][:], ones_w[:, 0:64],
                             start=True, stop=True)

            # ---- gold-path gathers (gpsimd; overlap with the scan) -------
            emi_t = cpool.tile([128, gcols], i32, tag="emi")
            tri_t = cpool.tile([128, gcols], i32, tag="tri")
            si_t = cpool.tile([BS, 1], i32, tag="si")
            ei_t = cpool.tile([BS, 1], i32, tag="ei")
            nc.sync.dma_start(emi_t[:], emidx[:])
            nc.sync.dma_start(tri_t[:], tridx[:])
            nc.sync.dma_start(si_t[:], sidx[:])
            nc.sync.dma_start(ei_t[:], eidx[:])

            emg = cpool.tile([128, gcols], f32, tag="emg")
            trg = cpool.tile([128, gcols], f32, tag="trg")
            sg = cpool.tile([BS, 1], f32, tag="sg")
            eg = cpool.tile([BS, 1], f32, tag="eg")
            nc.gpsimd.indirect_dma_start(
                emg[:], None,
                lg[:].rearrange("b t j -> (b t j)")[None, :],
                bass.IndirectOffsetOnAxis(ap=emi_t[:], axis=1))
            nc.gpsimd.indirect_dma_start(
                trg[:], None,
                Pm[:].rearrange("a b -> (a b)")[None, :],
                bass.IndirectOffsetOnAxis(ap=tri_t[:], axis=1))
            nc.gpsimd.indirect_dma_start(
                sg[:], None, Sv[:],
                bass.IndirectOffsetOnAxis(ap=si_t[:], axis=1))
            nc.gpsimd.indirect_dma_start(
                eg[:], None, Ev[:],
                bass.IndirectOffsetOnAxis(ap=ei_t[:], axis=1))
            # on-chip constant masks (gpsimd iota + DVE compare at the end)
            bd_i = cpool.tile([128, BS], i32, tag="bdi")
            id_i = cpool.tile([BS, BS], i32, tag="idi")
            pm_i = cpool.tile([128, 1], i32, tag="pmi")
            nc.gpsimd.iota(bd_i[:], [[-16, BS]], channel_multiplier=1)
            nc.gpsimd.iota(id_i[:], [[-1, BS]], channel_multiplier=1)
            nc.gpsimd.iota(pm_i[:], [[0, 1]], channel_multiplier=1)

            # ---- the scan ------------------------------------------------
            a_cur = [None, None]
            for c in range(ch):
                lraw = lraw_pool.tile([128, 256], f32)
                src = lg[:].rearrange("b t j -> t b j")[c * CT:(c + 1) * CT]
                nc.sync.dma_start(lraw[:], src)
                lexp = lexp_pool.tile([128, 256], bf16)
                nc.scalar.activation(lexp[:], lraw[:], AF.Exp, bias=cshift[:])
                emt = [emt_pool.tile([128, 128], bf16, tag=f"emt{k}",
                                     name=f"emt{k}") for k in range(2)]
                for k in range(2):
                    nc.scalar.dma_start_transpose(
                        emt[k][:], lexp[:, k * 128:(k + 1) * 128])
                # single-wait join: DVE observes the transpose DMAs here so
                # the per-step multiplies only wait on PE
                for k in range(2):
                    tch = tpool.tile([1, 1], bf16, tag="tch")
                    nc.vector.tensor_copy(tch[:], emt[k][0:1, 0:1])

                for r in range(CT):
                    t = c * CT + r
                    sl = (slice(None), slice(r * BS, (r + 1) * BS))
                    if t == 0:
                        for k in range(2):
                            a0 = a_pool.tile([128, BS], bf16, tag=f"a{k}",
                                             name=f"a0{k}")
                            nc.vector.tensor_scalar_mul(
                                a0[:], emt[k][sl], expS[k][:])
                            a_cur[k] = a0
                        continue

                    ps = [ps_pool.tile([128, BS], f32, tag=f"ps{j}",
                                       name=f"ps{j}") for j in range(2)]
                    for j in range(2):
                        nc.tensor.matmul(ps[j][:], PB[0][j][:], a_cur[0][:],
                                         start=True, stop=False)
                        nc.tensor.matmul(ps[j][:], PB[1][j][:], a_cur[1][:],
                                         start=False, stop=True)
                    na = [None, None]
                    for k in range(2):
                        na[k] = a_pool.tile([128, BS], bf16, tag=f"a{k}",
                                            name=f"na{k}")
                        nc.vector.tensor_mul(na[k][:], ps[k][:], emt[k][sl])

                    if t % RESC == 0:
                        sb = junk_pool.tile([128, BS], f32, tag="junk", name="sb")
                        nc.tensor.matmul(sb[:], ones_w[:], na[0][:],
                                         start=True, stop=False)
                        nc.tensor.matmul(sb[:], ones_w[:], na[1][:],
                                         start=False, stop=True)
                        rsb = rs_pool.tile([128, BS], f32, tag="rsb")
                        nc.vector.reciprocal(rsb[:], sb[:])
                        # ln(1/sigma) from rsb keeps sb single-consumer (DVE)
                        lns = rs_pool.tile([1, 8], f32, tag="lns")
                        nc.scalar.activation(lns[:], rsb[0:1, :], AF.Ln)
                        nc.vector.tensor_sub(acc_log[:], acc_log[:], lns[:])
                        for k in range(2):
                            sa = a_pool.tile([128, BS], bf16, tag=f"a{k}",
                                             name=f"sa{k}")
                            nc.vector.tensor_mul(sa[:], na[k][:], rsb[:])
                            a_cur[k] = sa
                    else:
                        a_cur = na

            # ---- finale: log_Z -------------------------------------------
            fa = [None, None]
            for k in range(2):
                fa[k] = a_pool.tile([128, BS], bf16, tag=f"a{k}", name=f"fa{k}")
                nc.vector.tensor_scalar_mul(fa[k][:], a_cur[k][:], expE[k][:])
            zps = fin_pool.tile([1, 8], f32, tag="fin")
            nc.tensor.matmul(zps[:], ones_w[:, 0:1], fa[0][:],
                             start=True, stop=False)
            nc.tensor.matmul(zps[:], ones_w[:, 0:1], fa[1][:],
                             start=False, stop=True)
            lnz = spool.tile([1, 8], f32, tag="lnz")
            nc.scalar.activation(lnz[:], zps[:], AF.Ln)
            zvec = spool.tile([1, 8], f32, tag="zvec")
            nc.vector.tensor_add(zvec[:], lnz[:], acc_log[:])

            # ---- finale: gold score --------------------------------------
            # single-wait joins for the four gather results
            for gi, g in enumerate((emg, trg, sg, eg)):
                tch = tpool.tile([1, 1], f32, tag="tchg", name=f"tchg{gi}")
                nc.vector.tensor_copy(tch[:], g[0:1, 0:1])
            # masks: bd[p,b] = (p//16 == b); id[p,b] = (p == b); pm = p%16 != 15
            bdm_t = cpool.tile([128, BS], f32, tag="bdm")
            idm_t = cpool.tile([BS, BS], f32, tag="idm")
            pm_t = cpool.tile([128, 1], f32, tag="pm")
            tmpi = cpool.tile([128, BS], i32, tag="tmpi")
            nc.vector.tensor_scalar(tmpi[:], bd_i[:], -16, None, ALU.bitwise_and)
            nc.vector.tensor_scalar(bdm_t[:], tmpi[:], 0, None, ALU.is_equal)
            nc.vector.tensor_scalar(idm_t[:], id_i[:], 0, None, ALU.is_equal)
            tmpp = cpool.tile([128, 1], i32, tag="tmpp")
            nc.vector.tensor_scalar(tmpp[:], pm_i[:], 15, None, ALU.bitwise_and)
            nc.vector.tensor_scalar(pm_t[:], tmpp[:], 15, None, ALU.not_equal)

            # pad slots (p%16==15, last col) gathered P[0,0]; mask them out
            nc.vector.tensor_mul(trg[:, gcols - 1:gcols],
                                 trg[:, gcols - 1:gcols], pm_t[:])
            emsum = spool.tile([128, 1], f32, tag="emsum")
            trsum = spool.tile([128, 1], f32, tag="trsum")
            nc.vector.tensor_reduce(emsum[:], emg[:], AX.X, ALU.add)
            nc.vector.tensor_reduce(trsum[:], trg[:], AX.X, ALU.add)
            gsum = spool.tile([128, 1], f32, tag="gsum")
            nc.vector.tensor_add(gsum[:], emsum[:], trsum[:])
            bd_ps = fin_pool.tile([1, BS], f32, tag="fin")
            nc.tensor.matmul(bd_ps[:], gsum[:], bdm_t[:], start=True, stop=True)
            seg = spool.tile([BS, 1], f32, tag="seg")
            nc.vector.tensor_add(seg[:], sg[:], eg[:])
            se_ps = fin_pool.tile([1, BS], f32, tag="fin")
            nc.tensor.matmul(se_ps[:], seg[:], idm_t[:], start=True, stop=True)

            nv = spool.tile([1, BS], f32, tag="nv")
            nc.vector.tensor_sub(nv[:], zvec[:], bd_ps[:])
            nc.vector.tensor_sub(nv[:], nv[:], se_ps[:])
            red = spool.tile([1, 1], f32, tag="red")
            nc.vector.tensor_reduce(red[:], nv[:], AX.X, ALU.add)
            nc.sync.dma_start(out[:], red[:])

    nc.finalize()
    return nc


def gold_indices(labels, t_steps=T):
    """Per-core gather indices. labels: [BS, t_steps] int array."""
    gcols = t_steps // 16
    emi = np.zeros((128, gcols), np.int32)
    tri = np.zeros((128, gcols), np.int32)  # pad -> P[0,0], masked on-chip
    for b in range(BS):
        for g in range(16):
            for col in range(gcols):
                t = g * gcols + col
                p = b * 16 + g
                emi[p, col] = (b * t_steps + t) * V + labels[b, t]
                if t < t_steps - 1:
                    tri[p, col] = labels[b, t] * V + labels[b, t + 1]
    si = labels[:, 0].astype(np.int32).reshape(BS, 1)
    ei = labels[:, t_steps - 1].astype(np.int32).reshape(BS, 1)
    return emi, tri, si, ei


def make_in_maps(logits, labels, P, S, E, t_steps=T):
    in_maps = []
    for ci in range(NCORES):
        bsl = slice(ci * BS, (ci + 1) * BS)
        emi, tri, si, ei = gold_indices(labels[bsl], t_steps)
        in_maps.append({
            "lg": np.ascontiguousarray(logits[bsl], np.float32),
            "Pm": np.ascontiguousarray(P, np.float32),
            "Sv": np.ascontiguousarray(S.reshape(1, V), np.float32),
            "Ev": np.ascontiguousarray(E.reshape(1, V), np.float32),
            "emidx": emi, "tridx": tri, "sidx": si, "eidx": ei,
        })
    return in_maps


_NC_CACHE = {}


def kernel(logits, labels, P, S, E):
    from concourse import bass_utils
    if "nc" not in _NC_CACHE:
        _NC_CACHE["nc"] = build(T)
    nc = _NC_CACHE["nc"]
    in_maps = make_in_maps(np.asarray(logits), np.asarray(labels),
                           np.asarray(P), np.asarray(S), np.asarray(E))
    rr = bass_utils.run_bass_kernel_spmd(nc, in_maps, core_ids=list(range(NCORES)))
    _NC_CACHE["last_rr"] = rr
    tot = np.float32(0.0)
    for r in rr.results:
        tot += np.float32(r["out"].reshape(-1)[0])
    return (tot / np.float32(B)).reshape(1).astype(np.float32)


# revision 19
# speedup vs baseline: 1.0064x; 1.0064x over previous
"""CRF loss (forward-algorithm NLL) on 8 Trainium2 NeuronCores.

Strategy: data-parallel over batch (8 sequences per core). The T-step
log-alpha recurrence runs in the exp domain so each step is a plain
matmul against exp(P) on the TensorEngine:

    a_t[j,b] = em_t[j,b] * sum_i expP[i,j] * a_{t-1}[i,b]

Emissions carry a constant shift exp(logit - ln(256*e)) so the state
magnitude stays near 1; an exact per-8-step rescale by the column sum
(via a ones-matmul, tracked in log space) absorbs the drift. The [j,b]
layout is preserved step to step (matmul output partitions = next
contraction partitions), so the scan needs no per-step transposes.
Emissions are exp'ed in bulk on the Scalar engine and transposed
chunkwise with DMA transpose into per-chunk fresh SBUF tiles (no slot
recycling -> every DMA carries at most the single sync-wait the ISA
allows). The gold-path score uses indirect-DMA gathers with host-
computed flat indices; cross-engine joins go through single-wait
"touch" ops so no instruction ever needs two semaphore waits.
"""

import os
import sys

import numpy as np

sys.path.insert(0, "/opt/trn_rl_repo")
os.environ.setdefault("MYCRO_LOCAL_CACHE", "1")

import concourse.bass as bass
import concourse.bacc as bacc
import concourse.mybir as mybir
from concourse.tile import TileContext

B, T, V = 64, 1024, 256
NCORES = 8
BS = B // NCORES          # 8 sequences per core
CT = 16                   # timesteps per emission chunk
RESC = 64                 # rescale period (steps)
C_SHIFT = 6.545177444479562  # ln(256*e); cancels expected per-step growth

f32 = mybir.dt.float32
fp8 = mybir.dt.float8e4
bf16 = mybir.dt.bfloat16
i32 = mybir.dt.int32
AF = mybir.ActivationFunctionType
ALU = mybir.AluOpType
AX = mybir.AxisListType


def build(t_steps=T):
    ch = t_steps // CT            # emission chunks
    gcols = t_steps // 16         # gather columns (16 groups per seq)

    nc = bacc.Bacc("TRN2")
    lg = nc.dram_tensor("lg", [BS, t_steps, V], f32, kind="ExternalInput")
    Pm = nc.dram_tensor("Pm", [V, V], f32, kind="ExternalInput")
    Sv = nc.dram_tensor("Sv", [1, V], f32, kind="ExternalInput")
    Ev = nc.dram_tensor("Ev", [1, V], f32, kind="ExternalInput")
    emidx = nc.dram_tensor("emidx", [128, gcols], i32, kind="ExternalInput")
    tridx = nc.dram_tensor("tridx", [128, gcols], i32, kind="ExternalInput")
    sidx = nc.dram_tensor("sidx", [BS, 1], i32, kind="ExternalInput")
    eidx = nc.dram_tensor("eidx", [BS, 1], i32, kind="ExternalInput")
    out = nc.dram_tensor("out", [1, 1], f32, kind="ExternalOutput")

    with TileContext(nc) as tc:
        with (
            tc.tile_pool(name="const", bufs=1) as cpool,
            tc.tile_pool(name="lraw", bufs=16) as lraw_pool,
            tc.tile_pool(name="lexp", bufs=16) as lexp_pool,
            tc.tile_pool(name="emt", bufs=16) as emt_pool,
            tc.tile_pool(name="a", bufs=t_steps + 140) as a_pool,
            tc.tile_pool(name="small", bufs=4) as spool,
            tc.tile_pool(name="rs", bufs=4) as rs_pool,
            tc.tile_pool(name="tch", bufs=2) as tpool,
            tc.tile_pool(name="ps", bufs=2, space="PSUM") as ps_pool,
            tc.tile_pool(name="sb", bufs=1, space="PSUM") as sb_pool,
            tc.tile_pool(name="fin", bufs=2, space="PSUM") as fin_pool,
            tc.tile_pool(name="junk", bufs=1, space="PSUM") as junk_pool,
        ):
            # ---- preamble: constants -------------------------------------
            praw = [cpool.tile([128, 256], f32, tag=f"praw{k}", name=f"praw{k}")
                    for k in range(2)]
            for k in range(2):
                nc.sync.dma_start(praw[k][:], Pm[k * 128:(k + 1) * 128, :])
            # PB[k][j] = exp(P[i-half k, j-half j]) in bf16
            PB = [[cpool.tile([128, 128], fp8, tag=f"pb{k}{j}", name=f"pb{k}{j}")
                   for j in range(2)] for k in range(2)]
            for k in range(2):
                for j in range(2):
                    nc.scalar.activation(
                        PB[k][j][:], praw[k][:, j * 128:(j + 1) * 128], AF.Exp)
            # emission shift, produced on ACT so the chunk exps join on one sem
            cshift = cpool.tile([128, 1], f32, tag="cshift")
            nc.scalar.activation(cshift[:], praw[0][:, 0:1], AF.Copy,
                                 bias=-C_SHIFT, scale=0.0)

            # exp(S), exp(E) as per-partition scalars [128,1] x2 (ACT)
            expS = [cpool.tile([128, 1], f32, tag=f"es{k}", name=f"es{k}")
                    for k in range(2)]
            expE = [cpool.tile([128, 1], f32, tag=f"ee{k}", name=f"ee{k}")
                    for k in range(2)]
            for k in range(2):
                svk = Sv[:].rearrange("a (p f) -> a p f", f=1)[0, k * 128:(k + 1) * 128]
                evk = Ev[:].rearrange("a (p f) -> a p f", f=1)[0, k * 128:(k + 1) * 128]
                tmpS = spool.tile([128, 1], f32, tag="tmpv")
                tmpE = spool.tile([128, 1], f32, tag="tmpv")
                nc.sync.dma_start(tmpS[:], svk)
                nc.sync.dma_start(tmpE[:], evk)
                nc.scalar.activation(expS[k][:], tmpS[:], AF.Exp)
                nc.scalar.activation(expE[k][:], tmpE[:], AF.Exp)

            ones_w = cpool.tile([128, 128], bf16, tag="ones")
            nc.vector.memset(ones_w[:], 1.0)
            acc_log = cpool.tile([1, 8], f32, tag="acc")
            nc.vector.memset(acc_log[:], float(t_steps) * C_SHIFT)

            # warm-up matmul: advances PE's view of ACT past the PB exps so
            # the scan matmuls each carry a single (DVE) wait
            warm_ps = junk_pool.tile([128, 64], f32, tag="junk")
            nc.tensor.matmul(warm_ps[:], PB[0][0][:], ones_w[:, 0:64],
                             start=True, stop=True)

            # ---- gold-path gathers (gpsimd; overlap with the scan) -------
            emi_t = cpool.tile([128, gcols], i32, tag="emi")
            tri_t = cpool.tile([128, gcols], i32, tag="tri")
            si_t = cpool.tile([BS, 1], i32, tag="si")
            ei_t = cpool.tile([BS, 1], i32, tag="ei")
            nc.sync.dma_start(emi_t[:], emidx[:])
            nc.sync.dma_start(tri_t[:], tridx[:])
            nc.sync.dma_start(si_t[:], sidx[:])
            nc.sync.dma_start(ei_t[:], eidx[:])

            emg = cpool.tile([128, gcols], f32, tag="emg")
            trg = cpool.tile([128, gcols], f32, tag="trg")
            sg = cpool.tile([BS, 1], f32, tag="sg")
            eg = cpool.tile([BS, 1], f32, tag="eg")
            nc.gpsimd.indirect_dma_start(
                emg[:], None,
                lg[:].rearrange("b t j -> (b t j)")[None, :],
                bass.IndirectOffsetOnAxis(ap=emi_t[:], axis=1))
            nc.gpsimd.indirect_dma_start(
                trg[:], None,
                Pm[:].rearrange("a b -> (a b)")[None, :],
                bass.IndirectOffsetOnAxis(ap=tri_t[:], axis=1))
            nc.gpsimd.indirect_dma_start(
                sg[:], None, Sv[:],
                bass.IndirectOffsetOnAxis(ap=si_t[:], axis=1))
            nc.gpsimd.indirect_dma_start(
                eg[:], None, Ev[:],
                bass.IndirectOffsetOnAxis(ap=ei_t[:], axis=1))
            # on-chip constant masks (gpsimd iota + DVE compare at the end)
            bd_i = cpool.tile([128, BS], i32, tag="bdi")
            id_i = cpool.tile([BS, BS], i32, tag="idi")
            pm_i = cpool.tile([128, 1], i32, tag="pmi")
            nc.gpsimd.iota(bd_i[:], [[-16, BS]], channel_multiplier=1)
            nc.gpsimd.iota(id_i[:], [[-1, BS]], channel_multiplier=1)
            nc.gpsimd.iota(pm_i[:], [[0, 1]], channel_multiplier=1)

            # ---- the scan ------------------------------------------------
            a_cur = [None, None]
            for c in range(ch):
                lraw = lraw_pool.tile([128, 256], f32)
                src = lg[:].rearrange("b t j -> t b j")[c * CT:(c + 1) * CT]
                nc.sync.dma_start(lraw[:], src)
                lexp = lexp_pool.tile([128, 256], bf16)
                nc.scalar.activation(lexp[:], lraw[:], AF.Exp, bias=cshift[:])
                emt = [emt_pool.tile([128, 128], bf16, tag=f"emt{k}",
                                     name=f"emt{k}") for k in range(2)]
                for k in range(2):
                    nc.scalar.dma_start_transpose(
                        emt[k][:], lexp[:, k * 128:(k + 1) * 128])
                # single-wait join: DVE observes the transpose DMAs here so
                # the per-step multiplies only wait on PE
                for k in range(2):
                    tch = tpool.tile([1, 1], bf16, tag="tch")
                    nc.vector.tensor_copy(tch[:], emt[k][0:1, 0:1])

                for r in range(CT):
                    t = c * CT + r
                    sl = (slice(None), slice(r * BS, (r + 1) * BS))
                    if t == 0:
                        for k in range(2):
                            a0 = a_pool.tile([128, BS], bf16, tag=f"a{k}",
                                             name=f"a0{k}")
                            nc.vector.tensor_scalar_mul(
                                a0[:], emt[k][sl], expS[k][:])
                            a_cur[k] = a0
                        continue

                    ps = [ps_pool.tile([128, BS], f32, tag=f"ps{j}",
                                       name=f"ps{j}") for j in range(2)]
                    for j in range(2):
                        nc.tensor.matmul(ps[j][:], PB[0][j][:], a_cur[0][:],
                                         start=True, stop=False)
                        nc.tensor.matmul(ps[j][:], PB[1][j][:], a_cur[1][:],
                                         start=False, stop=True)
                    na = [None, None]
                    for k in range(2):
                        na[k] = a_pool.tile([128, BS], bf16, tag=f"a{k}",
                                            name=f"na{k}")
                        nc.vector.tensor_mul(na[k][:], ps[k][:], emt[k][sl])

                    if t % RESC == 0:
                        sb = sb_pool.tile([128, BS], f32)
                        nc.tensor.matmul(sb[:], ones_w[:], na[0][:],
                                         start=True, stop=False)
                        nc.tensor.matmul(sb[:], ones_w[:], na[1][:],
                                         start=False, stop=True)
                        rsb = rs_pool.tile([128, BS], f32, tag="rsb")
                        nc.vector.reciprocal(rsb[:], sb[:])
                        # ln(1/sigma) from rsb keeps sb single-consumer (DVE)
                        lns = rs_pool.tile([1, 8], f32, tag="lns")
                        nc.scalar.activation(lns[:], rsb[0:1, :], AF.Ln)
                        nc.vector.tensor_sub(acc_log[:], acc_log[:], lns[:])
                        for k in range(2):
                            sa = a_pool.tile([128, BS], bf16, tag=f"a{k}",
                                             name=f"sa{k}")
                            nc.vector.tensor_mul(sa[:], na[k][:], rsb[:])
                            a_cur[k] = sa
                    else:
                        a_cur = na

            # ---- finale: log_Z -------------------------------------------
            fa = [None, None]
            for k in range(2):
                fa[k] = a_pool.tile([128, BS], bf16, tag=f"a{k}", name=f"fa{k}")
                nc.vector.tensor_scalar_mul(fa[k][:], a_cur[k][:], expE[k][:])
            zps = fin_pool.tile([1, 8], f32, tag="fin")
            nc.tensor.matmul(zps[:], ones_w[:, 0:1], fa[0][:],
                             start=True, stop=False)
            nc.tensor.matmul(zps[:], ones_w[:, 0:1], fa[1][:],
                             start=False, stop=True)
            lnz = spool.tile([1, 8], f32, tag="lnz")
            nc.scalar.activation(lnz[:], zps[:], AF.Ln)
            zvec = spool.tile([1, 8], f32, tag="zvec")
            nc.vector.tensor_add(zvec[:], lnz[:], acc_log[:])

            # ---- finale: gold score --------------------------------------
            # single-wait joins for the four gather results
            for gi, g in enumerate((emg, trg, sg, eg)):
                tch = tpool.tile([1, 1], f32, tag="tchg", name=f"tchg{gi}")
                nc.vector.tensor_copy(tch[:], g[0:1, 0:1])
            # masks: bd[p,b] = (p//16 == b); id[p,b] = (p == b); pm = p%16 != 15
            bdm_t = cpool.tile([128, BS], f32, tag="bdm")
            idm_t = cpool.tile([BS, BS], f32, tag="idm")
            pm_t = cpool.tile([128, 1], f32, tag="pm")
            tmpi = cpool.tile([128, BS], i32, tag="tmpi")
            nc.vector.tensor_scalar(tmpi[:], bd_i[:], -16, None, ALU.bitwise_and)
            nc.vector.tensor_scalar(bdm_t[:], tmpi[:], 0, None, ALU.is_equal)
            nc.vector.tensor_scalar(idm_t[:], id_i[:], 0, None, ALU.is_equal)
            tmpp = cpool.tile([128, 1], i32, tag="tmpp")
            nc.vector.tensor_scalar(tmpp[:], pm_i[:], 15, None, ALU.bitwise_and)
            nc.vector.tensor_scalar(pm_t[:], tmpp[:], 15, None, ALU.not_equal)

            # pad slots (p%16==15, last col) gathered P[0,0]; mask them out
            nc.vector.tensor_mul(trg[:, gcols - 1:gcols],
                                 trg[:, gcols - 1:gcols], pm_t[:])
            emsum = spool.tile([128, 1], f32, tag="emsum")
            trsum = spool.tile([128, 1], f32, tag="trsum")
            nc.vector.tensor_reduce(emsum[:], emg[:], AX.X, ALU.add)
            nc.vector.tensor_reduce(trsum[:], trg[:], AX.X, ALU.add)
            gsum = spool.tile([128, 1], f32, tag="gsum")
            nc.vector.tensor_add(gsum[:], emsum[:], trsum[:])
            bd_ps = fin_pool.tile([1, BS], f32, tag="fin")
            nc.tensor.matmul(bd_ps[:], gsum[:], bdm_t[:], start=True, stop=True)
            seg = spool.tile([BS, 1], f32, tag="seg")
            nc.vector.tensor_add(seg[:], sg[:], eg[:])
            se_ps = fin_pool.tile([1, BS], f32, tag="fin")
            nc.tensor.matmul(se_ps[:], seg[:], idm_t[:], start=True, stop=True)

            nv = spool.tile([1, BS], f32, tag="nv")
            nc.vector.tensor_sub(nv[:], zvec[:], bd_ps[:])
            nc.vector.tensor_sub(nv[:], nv[:], se_ps[:])
            red = spool.tile([1, 1], f32, tag="red")
            nc.vector.tensor_reduce(red[:], nv[:], AX.X, ALU.add)
            nc.sync.dma_start(out[:], red[:])

    nc.finalize()
    return nc


def gold_indices(labels, t_steps=T):
    """Per-core gather indices. labels: [BS, t_steps] int array."""
    gcols = t_steps // 16
    emi = np.zeros((128, gcols), np.int32)
    tri = np.zeros((128, gcols), np.int32)  # pad -> P[0,0], masked on-chip
    for b in range(BS):
        for g in range(16):
            for col in range(gcols):
                t = g * gcols + col
                p = b * 16 + g
                emi[p, col] = (b * t_steps + t) * V + labels[b, t]
                if t < t_steps - 1:
                    tri[p, col] = labels[b, t] * V + labels[b, t + 1]
    si = labels[:, 0].astype(np.int32).reshape(BS, 1)
    ei = labels[:, t_steps - 1].astype(np.int32).reshape(BS, 1)
    return emi, tri, si, ei


def make_in_maps(logits, labels, P, S, E, t_steps=T):
    in_maps = []
    for ci in range(NCORES):
        bsl = slice(ci * BS, (ci + 1) * BS)
        emi, tri, si, ei = gold_indices(labels[bsl], t_steps)
        in_maps.append({
            "lg": np.ascontiguousarray(logits[bsl], np.float32),
            "Pm": np.ascontiguousarray(P, np.float32),
            "Sv": np.ascontiguousarray(S.reshape(1, V), np.float32),
            "Ev": np.ascontiguousarray(E.reshape(1, V), np.float32),
            "emidx": emi, "tridx": tri, "sidx": si, "eidx": ei,
        })
    return in_maps


_NC_CACHE = {}


def kernel(logits, labels, P, S, E):
    from concourse import bass_utils
    if "nc" not in _NC_CACHE:
        _NC_CACHE["nc"] = build(T)
    nc = _NC_CACHE["nc"]
    in_maps = make_in_maps(np.asarray(logits), np.asarray(labels),
                           np.asarray(P), np.asarray(S), np.asarray(E))
    rr = bass_utils.run_bass_kernel_spmd(nc, in_maps, core_ids=list(range(NCORES)))
    _NC_CACHE["last_rr"] = rr
    tot = np.float32(0.0)
    for r in rr.results:
        tot += np.float32(r["out"].reshape(-1)[0])
    return (tot / np.float32(B)).reshape(1).astype(np.float32)
